# revision 62
# baseline (speedup 1.0000x reference)
"""GQA attention (B=2, L=2048, D=2048, Hq=32, Hkv=8, hd=64) on 8 TRN2 cores.

Tensor-parallel over heads: core c owns q heads 4c..4c+3 and kv head c.
Each core computes a partial output (wo input-dim shard); host sums partials.

Design: all-bf16 matmul pipeline; merged QKV projection (3 full-width
128-row M-blocks per contraction chunk, nt-pair x tiles, deep DMA
prefetch on dual queues); RoPE on DVE via stream_shuffle with
host-precomputed cos/sin tables; V transposed via DMA-transpose straight
into vA; causal trapezoid tiling (128-key granularity inside 512-query
blocks); no-max softmax with ones-augmented V for the denominator;
fast-approx reciprocal; a global cross-head software pipeline (scores
lead AV by 4 steps, exp on ACT is the pacing stage) with out-projection
matmul units sprinkled into PE slack.

All attention matmuls use full 128x128 stationary tiles (k zero-padded
into kTe/kTo, vA chunks padded to 128 cols) so LDWEIGHTS goes to the
background weight plane and overlaps the previous matmul's stream.
Score tiles are paired into 2-bank PSUM tiles so one exp instruction
covers 1024 columns; the softmax denominator row is broadcast on the
otherwise-idle GPSIMD engine.

Per-core layouts (feature-on-partition):
  xT      [2048, 4096]   x transposed bf16 (shared by all cores)
  wqkv_t  [2048, 384]    [wq shard (perm) | wk shard (perm) | wv shard] ^T
  wo_t    [256, 2048]    wo columns shard, transposed
  cos/sin [128, 2048]    rope tables, 32-row periodic, sign baked in sin
  mask    [128, 128]     causal tile (keep iff col >= row)
  outT    [2048, 4096]   partial output bf16 (host: sum, T, reshape)
"""
import ml_dtypes
import numpy as np
from contextlib import ExitStack

import concourse.bass as bass
import concourse.mybir as mybir
import concourse.tile as tile
from concourse import bacc
from concourse.bass_utils import run_bass_kernel_spmd

F32 = mybir.dt.float32
BF16 = mybir.dt.bfloat16
I32 = mybir.dt.int32
AF = mybir.ActivationFunctionType
ALU = mybir.AluOpType

B, L, D = 2, 2048, 2048
HQ, HKV, HD = 32, 8, 64
NCORES = 8
HL = HQ // NCORES          # 4 q heads per core
DQ = HL * HD               # 256 local q features
T = B * L                  # 4096 tokens
NB = 512                   # token block
NT = T // NB               # 8 token blocks
KC = D // 128              # 16 contraction chunks
ROPE_BASE = 10000.0
SCALE = 1.0 / np.sqrt(HD)

# stream_shuffle permutes per-partition within each 32-block (replicated
# every 32 rows): swap 16-row halves -> rope partner exchange
SHUF = [j ^ 16 for j in range(32)]
VCH = 128                  # vA chunk stride (full-width stationary tiles)

_CACHE = {}


def _build_module():
    nc = bacc.Bacc("TRN2", target_bir_lowering=False, debug=False,
                   num_devices=NCORES)

    d_xT = nc.dram_tensor("xT", [D, T], BF16, kind="ExternalInput").ap()
    d_wqkv = nc.dram_tensor("wqkv_t", [D, 384], BF16, kind="ExternalInput").ap()
    d_wo = nc.dram_tensor("wo_t", [DQ, D], BF16, kind="ExternalInput").ap()
    d_cos = nc.dram_tensor("cos_t", [128, L], BF16, kind="ExternalInput").ap()
    d_sin = nc.dram_tensor("sin_t", [128, L], BF16, kind="ExternalInput").ap()
    d_mask = nc.dram_tensor("mask", [128, 128], BF16, kind="ExternalInput").ap()
    d_eye = nc.dram_tensor("eye64", [64, 64], BF16, kind="ExternalInput").ap()
    d_out = nc.dram_tensor("outT", [D, T], BF16, kind="ExternalOutput").ap()

    with tile.TileContext(nc) as tc, ExitStack() as ctx, \
         nc.allow_low_precision(reason="bf16 matmul pipeline"):
        _kernel(tc, ctx, d_xT, d_wqkv, d_wo, d_cos, d_sin, d_mask, d_eye,
                d_out)

    nc.compile()
    return nc


def _kernel(tc, ctx, d_xT, d_wqkv, d_wo, d_cos, d_sin, d_mask, d_eye, d_out,
            dump=None):
    nc = tc.nc

    wpool = ctx.enter_context(tc.tile_pool(name="weights", bufs=1))
    spool = ctx.enter_context(tc.tile_pool(name="state", bufs=1))

    # ---------------- persistent SBUF tensors ----------------
    wqkvT = wpool.tile([128, KC * 384], BF16, tag="wqkvT")   # 12KB/part
    woT = wpool.tile([128, 2 * D], BF16, tag="woT")          # 8KB
    c128 = wpool.tile([128, L], BF16, tag="c128")            # 4KB
    s128 = wpool.tile([128, L], BF16, tag="s128")            # 4KB
    maskA = wpool.tile([128, 128], BF16, tag="maskA")
    eye64 = wpool.tile([64, 64], BF16, tag="eye64")

    # weight-ish DMAs all on the sync queue (in kc order); x tiles go on
    # scalar/gpsimd for group 0, rotating over all three queues after
    nc.sync.dma_start(maskA[:], d_mask[:])
    nc.sync.dma_start(eye64[:], d_eye[:])
    for kc in range(KC):
        nc.sync.dma_start(wqkvT[:, kc * 384:(kc + 1) * 384],
                          d_wqkv[kc * 128:(kc + 1) * 128, :])
    nc.sync.dma_start(c128[:], d_cos[:])
    nc.sync.dma_start(s128[:], d_sin[:])
    for kc2 in range(2):
        nc.sync.dma_start(woT[:, kc2 * D:(kc2 + 1) * D],
                          d_wo[kc2 * 128:(kc2 + 1) * 128, :])

    # qT: [128, 2*T]; head pair p cols [p*T, (p+1)*T); even head rows 0:64,
    # odd head rows 64:128; within a head [even dims | odd dims].
    qT = spool.tile([128, 2 * T], BF16, tag="qT")            # 16KB
    # k zero-padded into full-height tiles: kTe rows 0:64 = k, 64:128 = 0;
    # kTo rows 0:64 = 0, 64:128 = k.  Full 128x128 stationaries let
    # LDWEIGHTS use the background weight plane (overlapped load).
    kTe = spool.tile([128, T], BF16, tag="kTe")              # 8KB
    kTo = spool.tile([128, T], BF16, tag="kTo")              # 8KB
    nc.vector.memset(kTe[:], 0.0)
    nc.vector.memset(kTo[:], 0.0)
    # v natural layout + ones column: chunk ch = cols [VCH*ch, VCH*ch+128)
    vA = spool.tile([128, 32 * VCH], BF16, tag="vA")         # 8KB
    nc.vector.memset(vA[:], 1.0)  # ones columns; data cols overwritten
    # attention output, transposed: head pair tiles, b-major columns
    atP = [spool.tile([128, T], BF16, tag=f"atP{p}", name=f"atP{p}")
           for p in range(2)]                                # 16KB

    # ---------------- RoPE (DVE) ------------------------------------------
    tpool = ctx.enter_context(tc.tile_pool(name="tmp", bufs=2))

    def rope(dst, rows, cols, l0):
        # y = x*cos + swap(x)*sgn*sin; swap = exchange 16-row halves per
        # 32-block (sign baked into the sin table on host)
        nr = rows.stop - rows.start
        xs_ = tpool.tile([128, NB], BF16, tag="ropeS")
        u = tpool.tile([128, NB], BF16, tag="ropeU")
        w = tpool.tile([128, NB], BF16, tag="ropeW")
        nc.vector.stream_shuffle(xs_[0:nr, :], dst[rows, cols], SHUF)
        nc.vector.tensor_mul(u[0:nr, :], dst[rows, cols],
                             c128[rows, l0:l0 + NB])
        nc.vector.tensor_mul(w[0:nr, :], xs_[0:nr, :],
                             s128[rows, l0:l0 + NB])
        nc.vector.tensor_add(dst[rows, cols], u[0:nr, :], w[0:nr, :])

    def rope_part(nt, part):
        b, l0 = nt // 4, (nt % 4) * NB
        if part < 2:
            c0 = part * T + b * L + l0
            rope(qT, slice(0, 128), slice(c0, c0 + NB), l0)
        else:
            cols = slice(b * L + l0, b * L + l0 + NB)
            rope(kTe, slice(0, 64), cols, l0)
            # odd-head copy of rope'd k via SBUF->SBUF DMA (no engine cost)
            nc.gpsimd.dma_start(kTo[64:128, cols], kTe[0:64, cols])

    def rope_nt(nt):
        for part in range(3):
            rope_part(nt, part)

    # ---------------- phase 1: merged QKV projection ----------------------
    with tc.tile_pool(name="xs", bufs=16) as xs, \
         tc.tile_pool(name="vst", bufs=2) as vst, \
         tc.tile_pool(name="pq0", bufs=2, space="PSUM") as pq0, \
         tc.tile_pool(name="pq1", bufs=2, space="PSUM") as pq1, \
         tc.tile_pool(name="pkv", bufs=2, space="PSUM") as pkv, \
         tc.tile_pool(name="ptp", bufs=2, space="PSUM") as ptp:
        for g in range(NT // 2):            # nt pairs share one x tile
            psq = [[pq0.tile([128, NB], F32, tag="psq0", name=f"psq{g}{s}0"),
                    pq1.tile([128, NB], F32, tag="psq1", name=f"psq{g}{s}1")]
                   for s in range(2)]
            pskv = [pkv.tile([128, NB], F32, tag="pskv", name=f"pskv{g}{s}")
                    for s in range(2)]
            # deep prefetch: issue the whole group's x DMAs up front
            xks = []
            for kc in range(KC):
                xk = xs.tile([128, 2 * NB], BF16, tag="xk", name=f"xk{g}_{kc}")
                # sync is backlogged with ~24us of weight transfers at the
                # start; once free it absorbs late-kc chunks of groups >= 1.
                # Odd groups lead on gpsimd: their first triggers would
                # otherwise queue behind the previous group's eviction
                # copies on the scalar/ACT engine queue
                pair = (nc.scalar, nc.gpsimd) if g % 2 == 0 \
                    else (nc.gpsimd, nc.scalar)
                if g == 0 or kc < 8:
                    q = pair[kc % 2]
                else:
                    q = (nc.sync, pair[0], pair[1])[kc % 3]
                q.dma_start(
                    xk[:], d_xT[kc * 128:(kc + 1) * 128,
                                g * 2 * NB:(g + 1) * 2 * NB])
                xks.append(xk)
            for kc in range(KC):
                st, sp = kc == 0, kc == KC - 1
                for s in range(2):
                    xm = xks[kc][:, s * NB:(s + 1) * NB]
                    for p in range(2):
                        nc.tensor.matmul(
                            psq[s][p][:],
                            wqkvT[:, kc * 384 + p * 128: kc * 384 + (p + 1) * 128],
                            xm, start=st, stop=sp)
                    nc.tensor.matmul(
                        pskv[s][:], wqkvT[:, kc * 384 + 256:(kc + 1) * 384],
                        xm, start=st, stop=sp)
            # evictions split ACT/DVE and ordered to match the next group's
            # matmul consumption (q0, q1, kv per sub); ropes go last so the
            # DVE queue is clear for the eviction copies
            for s in range(2):
                nt = 2 * g + s
                cols = slice(nt * NB, (nt + 1) * NB)
                nc.scalar.copy(qT[:, 0 * T + nt * NB: 0 * T + (nt + 1) * NB],
                               psq[s][0][:])
                nc.vector.tensor_copy(
                    qT[:, 1 * T + nt * NB: 1 * T + (nt + 1) * NB], psq[s][1][:])
                nc.scalar.copy(kTe[0:64, cols], pskv[s][0:64, :])
                vstage = vst.tile([64, NB], BF16, tag="vstage", name=f"vst{nt}")
                nc.vector.tensor_copy(vstage[:], pskv[s][64:128, :])
                # V transpose on PE (dma_start_transpose triggers cost
                # ~1.5us of descriptor generation on the issuing engine)
                for t4 in range(4):
                    ch = nt * 4 + t4
                    tp_ps = ptp.tile([128, 64], BF16, tag="tp",
                                     name=f"tp{nt}_{t4}")
                    nc.tensor.transpose(tp_ps[:], vstage[:, t4 * 128:(t4 + 1) * 128],
                                        eye64[:])
                    nc.vector.tensor_copy(vA[:, ch * VCH: ch * VCH + 64], tp_ps[:])
            for s in range(2):
                nt = 2 * g + s
                # the last group's ropes are deferred into phase 2 (their
                # k/q blocks are read no earlier than step ~200)
                if nt < 6:
                    rope_nt(nt)

    # ---------------- phase 2+3: attention + out-projection ---------------
    with tc.tile_pool(name="epool", bufs=7) as ep, \
         tc.tile_pool(name="npool", bufs=3) as npool, \
         tc.tile_pool(name="opool", bufs=6) as op, \
         tc.tile_pool(name="pst", bufs=2, space="PSUM") as pst, \
         tc.tile_pool(name="pot", bufs=2, space="PSUM") as pot, \
         tc.tile_pool(name="pout", bufs=2, space="PSUM") as pout:

        def emit_normA(state):
            # immediate DVE-only part: denominator -> bf16 reciprocal row
            p, rbase, b, qb, ot_ps = state
            # reciprocal_approx_fast cannot read PSUM (bit-twiddling path);
            # stage the denominator row to SBUF first (DVE: keeps the ACT
            # queue free for exp, which gates the PE score pipeline)
            den_sb = npool.tile([1, NB], F32, tag="den_sb")
            nc.vector.tensor_copy(den_sb[:], ot_ps[64:65, :])
            recip = npool.tile([1, NB], F32, tag="recip")
            nc.vector.reciprocal_approx_fast(recip[:], den_sb[:])
            recipb = npool.tile([1, NB], BF16, tag="recipb")
            nc.vector.tensor_copy(recipb[:], recip[:])
            ot_sb = npool.tile([64, NB], BF16, tag="ot_sb")
            nc.vector.tensor_copy(ot_sb[:], ot_ps[0:64, :])
            return recipb, ot_sb

        def emit_normB(state, recipb, ot_sb):
            # deferred part: GPSIMD broadcast + DVE normalize (recipb ready
            # by now; the numerator was staged to SBUF so ot PSUM is free)
            p, rbase, b, qb, ot_ps = state
            cols = slice(b * L + qb * NB, b * L + (qb + 1) * NB)
            denb = npool.tile([64, NB], BF16, tag="denb")
            nc.gpsimd.partition_broadcast(denb[:], recipb[:])
            nc.vector.tensor_mul(atP[p][rbase:rbase + 64, cols],
                                 ot_sb[:], denb[:])
            if p == 1 and rbase == 64:
                avail_out.extend((b * 4 + qb, mc) for mc in range(16))

        # global cross-head software pipeline: a continuous (qb, h, kb) step
        # stream so the exp stage never drains at head boundaries; out-proj
        # matmul units are sprinkled into the PE slack of exp-gated steps.
        LAG = 10
        pair_state = {}
        e_tiles = {}
        ot_tiles = {}
        avail_out = []
        pending_normB = []

        def emit_scores(i, b, qb, h, kb):
            j, half = divmod(i, 2)
            odd = h % 2
            p = h // 2
            kk = kTo if odd else kTe
            qcols = p * T + b * L + qb * NB
            diag_c = kb - 4 * qb
            col0 = 128 * diag_c if diag_c > 0 else 0
            if half == 0:
                st2 = pst.tile([128, 2 * NB], F32, tag="st", name=f"st{j}")
                e2 = ep.tile([128, 2 * NB], BF16, tag="e", name=f"e{j}")
                pair_state[j] = (st2, e2, [])
            st2, e2, metas = pair_state[j]
            base = half * NB
            nc.tensor.matmul(
                st2[:, base + col0: base + NB],
                kk[:, b * L + kb * 128: b * L + (kb + 1) * 128],
                qT[:, qcols + col0: qcols + NB],
                start=True, stop=True)
            metas.append((half, col0, diag_c))
            e_tiles[(b, qb, h, kb)] = (e2, base, col0)
            if half == 1:
                c0a, c1a = metas[0][1], metas[1][1]
                if c1a == 0:
                    # clean pair: one exp instruction over both banks
                    nc.scalar.activation(e2[:, c0a:], st2[:, c0a:],
                                         AF.Exp, scale=float(SCALE))
                else:
                    nc.scalar.activation(e2[:, c0a:NB], st2[:, c0a:NB],
                                         AF.Exp, scale=float(SCALE))
                    nc.scalar.activation(e2[:, NB + c1a:], st2[:, NB + c1a:],
                                         AF.Exp, scale=float(SCALE))
                for hf, cc, dc in metas:
                    if dc >= 0:
                        # only 128 cols over the diagonal tile need masking
                        nc.vector.tensor_mul(
                            e2[:, hf * NB + cc: hf * NB + cc + 128],
                            e2[:, hf * NB + cc: hf * NB + cc + 128], maskA[:])
                del pair_state[j]

        def emit_av(b, qb, h, kb):
            nkb = 4 * (qb + 1)
            e2, base, col0 = e_tiles.pop((b, qb, h, kb))
            if kb == 0:
                ot_tiles[(b, qb, h)] = pot.tile([128, NB], F32, tag="ot",
                                                name=f"ot{b}_{qb}_{h}")
            ot_ps = ot_tiles[(b, qb, h)]
            ch = b * 16 + kb
            nc.tensor.matmul(ot_ps[:, col0:], vA[:, ch * VCH: ch * VCH + VCH],
                             e2[:, base + col0: base + NB], start=(kb == 0),
                             stop=(kb == nkb - 1))
            if kb == nkb - 1:
                p, rbase = h // 2, 64 * (h % 2)
                state = (p, rbase, b, qb, ot_tiles.pop((b, qb, h)))
                recipb, ot_sb = emit_normA(state)
                pending_normB.append((state, recipb, ot_sb))

        def emit_out_unit(drain=False):
            nt, mc = avail_out.pop(0)
            po = pout.tile([128, NB], F32, tag="po")
            nc.tensor.matmul(po[:], woT[:, mc * 128:(mc + 1) * 128],
                             atP[0][:, nt * NB:(nt + 1) * NB],
                             start=True, stop=False)
            nc.tensor.matmul(po[:], woT[:, D + mc * 128: D + (mc + 1) * 128],
                             atP[1][:, nt * NB:(nt + 1) * NB],
                             start=False, stop=True)
            osb = op.tile([128, NB], BF16, tag="osb")
            # in-loop evictions on DVE (ACT is the exp pacer); during the
            # final drain ACT is idle and DVE would gate the PE
            if drain:
                nc.scalar.copy(osb[:], po[:])
            else:
                nc.vector.tensor_copy(osb[:], po[:])
            nc.sync.dma_start(
                d_out[mc * 128:(mc + 1) * 128, nt * NB:(nt + 1) * NB], osb[:])

        steps = [(b, qb, h, kb)
                 for b in range(B)
                 for qb in range(L // NB)
                 for h in range(HL)
                 for kb in range(4 * (qb + 1))]
        nsteps = len(steps)
        # last group's ropes, spread thinly through phase 2 (nt6 k needed
        # by step 208, nt7 k by step 256)
        DEFER_ROPE = {60: (6, 0), 90: (6, 1), 120: (6, 2),
                      150: (7, 0), 180: (7, 1), 210: (7, 2)}
        norm_ready = {}
        for i in range(nsteps + LAG):
            if i in DEFER_ROPE:
                rope_part(*DEFER_ROPE[i])
            if i < nsteps:
                emit_scores(i, *steps[i])
            if i >= LAG:
                emit_av(*steps[i - LAG])
            # deferred norm tail ~3 steps after its recip chain started
            if pending_normB:
                key = id(pending_normB[0])
                norm_ready.setdefault(key, i + 3)
                if i >= norm_ready[key]:
                    state, recipb, ot_sb = pending_normB.pop(0)
                    del norm_ready[key]
                    emit_normB(state, recipb, ot_sb)
            # ~2 out-proj units per 5 steps keeps PE slack filled; in qb==0
            # windows the trimmed diag score/AV matmuls leave the PE exp-
            # gated, so emit every step there.  Hold a few units in reserve
            # so the end-of-stream norm chain drains with PE work available.
            # No units before step ~37: their atP reads would head-of-line
            # block the PE on the transition-time DVE backlog
            in_qb0 = i < nsteps and steps[i][1] == 0
            pace = in_qb0 or i % 5 in (1, 3)
            if avail_out and i > 36 and ((pace and len(avail_out) > 6)
                                         or len(avail_out) > 24):
                emit_out_unit()
        while pending_normB:
            state, recipb, ot_sb = pending_normB.pop(0)
            emit_normB(state, recipb, ot_sb)
        while avail_out:
            emit_out_unit(drain=True)

    if dump is not None:
        for name, t in [("qT", qT), ("kTe", kTe), ("kTo", kTo), ("vA", vA),
                        ("atP0", atP[0]), ("atP1", atP[1]),
                        ("c128", c128), ("s128", s128),
                        ("maskA", maskA)]:
            if name not in dump:
                continue
            nc.sync.dma_start(dump[name][:], t[:])


ROPE_PERM = np.concatenate([np.arange(0, 32, 2), np.arange(1, 32, 2),
                            np.arange(32, 64, 2), np.arange(33, 64, 2)])


def _deinterleave_rows(w):
    # [H*64, D] -> per-head rows reordered so rope partners sit 16 apart
    # within each 32-row block: [e0..e15 | o0..o15 | e16..e31 | o16..o31]
    h = w.shape[0] // HD
    out = np.empty_like(w)
    for i in range(h):
        out[i * HD:(i + 1) * HD] = w[i * HD:(i + 1) * HD][ROPE_PERM]
    return out


def _rope_tables(pos_ids):
    half = HD // 2
    invf = (1.0 / (ROPE_BASE ** (np.arange(half, dtype=np.float32) / half)))
    # row r holds freq for lane (r%16) of half-block ((r%64)//32)
    idx = np.array([16 * ((r % 64) // 32) + (r % 16) for r in range(128)])
    ang = pos_ids.astype(np.float32)[None, :] * invf[idx][:, None]  # [128, L]
    sign = np.where((np.arange(128) % 32) < 16, -1.0, 1.0).astype(np.float32)
    cos_t = np.cos(ang).astype(ml_dtypes.bfloat16)
    sin_t = (np.sin(ang) * sign[:, None]).astype(ml_dtypes.bfloat16)
    return np.ascontiguousarray(cos_t), np.ascontiguousarray(sin_t)


def _prep_inputs(x, pos_ids, wq, wk, wv, wo):
    xT = np.ascontiguousarray(
        x.reshape(T, D).T).astype(ml_dtypes.bfloat16)
    cos_t, sin_t = _rope_tables(np.asarray(pos_ids))
    mask = np.ascontiguousarray(
        (np.arange(128)[None, :] >= np.arange(128)[:, None])
        .astype(ml_dtypes.bfloat16))
    eye64 = np.eye(64).astype(ml_dtypes.bfloat16)
    in_maps = []
    for c in range(NCORES):
        wq_c = _deinterleave_rows(wq[c * DQ:(c + 1) * DQ])
        wk_c = _deinterleave_rows(wk[c * HD:(c + 1) * HD])
        wv_c = wv[c * HD:(c + 1) * HD]
        wqkv = np.concatenate([wq_c, wk_c, wv_c], axis=0)   # [384, D]
        wo_c = wo[:, c * DQ:(c + 1) * DQ]
        in_maps.append({
            "xT": xT,
            "wqkv_t": np.ascontiguousarray(wqkv.T).astype(ml_dtypes.bfloat16),
            "wo_t": np.ascontiguousarray(wo_c.T).astype(ml_dtypes.bfloat16),
            "cos_t": cos_t,
            "sin_t": sin_t,
            "mask": mask,
            "eye64": eye64,
        })
    return in_maps


def kernel(x, pos_ids, wq, wk, wv, wo, _trace=False):
    x = np.asarray(x)
    if "nc" not in _CACHE:
        _CACHE["nc"] = _build_module()
    nc = _CACHE["nc"]
    in_maps = _prep_inputs(np.asarray(x, np.float32), np.asarray(pos_ids),
                           np.asarray(wq, np.float32), np.asarray(wk, np.float32),
                           np.asarray(wv, np.float32), np.asarray(wo, np.float32))
    res = run_bass_kernel_spmd(nc, in_maps, core_ids=list(range(NCORES)),
                               trace=_trace)
    _CACHE["last_results"] = res
    acc = np.zeros((D, T), np.float32)
    for r in res.results:
        acc += r["outT"].astype(np.float32)
    return np.ascontiguousarray(acc.T).reshape(B, L, D)
